# revision 14
# baseline (speedup 1.0000x reference)
"""GQA attention layer (32 Q heads / 8 KV heads / head_dim 128) with RoPE and
KV cache, tensor-parallel over heads across 8 TRN2 NeuronCores.

Sharding: core c owns Q heads 4c..4c+3 and KV head c.  Each core computes its
q/k/v projections, RoPE, causal flash-style attention against its KV-cache
shard, and a partial o_proj (input-dim sharded).  The o_proj partials are
summed host-side (the unshard step for partial-sum sharding), so the device
graph needs no collectives.

All matmuls run in bf16 with fp32 PSUM accumulation.  Layouts are arranged
host-side so every DMA is partition-major contiguous:
  - activations/weights are pre-transposed so the contraction dim (hid / head
    dim / key position) lands on SBUF partitions,
  - scores are computed transposed ([t_k, t_q]) so the exp'd scores feed the
    ctx matmul directly with no on-chip transposes of the prob matrix; the
    softmax denominator comes from an all-ones stationary matmul, and the
    bounded-logit range (|logit| < ~15) makes the max-subtraction unnecessary.
"""

import numpy as np
import ml_dtypes

import concourse.bass as bass
import concourse.tile as tile
from concourse import bacc, mybir
from concourse import bass_utils
from concourse.bass import ds

BF16 = mybir.dt.bfloat16
F32 = mybir.dt.float32
NP_BF16 = ml_dtypes.bfloat16

P = 128
B, Q, PAST = 2, 1024, 1024
T = Q + PAST
HID, NH, NKV, HD = 4096, 32, 8, 128
NCORES = 8
NHC = NH // NCORES          # q heads per core
HKT = HID // P              # hid contraction tiles
CH = 512                    # token chunk (matmul free dim)
NCH = (B * Q) // CH         # 4 chunks, b-major token order
SCALE = float(HD) ** -0.5
QWS = 4                     # qw DMA split granularity (ktiles per DMA tile)


def build_nc():
    nc = bacc.Bacc("TRN2", target_bir_lowering=False, debug=False)

    hid_d = nc.dram_tensor("hid_t", [P, HKT, B * Q], BF16, kind="ExternalInput").ap()
    qw_d = nc.dram_tensor("q_wt", [P, HKT, NHC * HD], BF16, kind="ExternalInput").ap()
    kw_d = nc.dram_tensor("k_wt", [P, HKT, HD], BF16, kind="ExternalInput").ap()
    vw_d = nc.dram_tensor("v_wt", [P, HKT, HD], BF16, kind="ExternalInput").ap()
    ow_d = nc.dram_tensor("o_wt", [P, NHC, HID], BF16, kind="ExternalInput").ap()
    pk_d = nc.dram_tensor("pk_t", [P, B, PAST], BF16, kind="ExternalInput").ap()
    pv_d = nc.dram_tensor("pv_t", [P, B, PAST // P, HD], BF16, kind="ExternalInput").ap()
    cos_d = nc.dram_tensor("cos2", [P, Q], F32, kind="ExternalInput").ap()
    sin_d = nc.dram_tensor("sinn", [P, Q], F32, kind="ExternalInput").ap()
    tri_d = nc.dram_tensor("tri", [P, P], BF16, kind="ExternalInput").ap()
    idn_d = nc.dram_tensor("idn", [P, P], BF16, kind="ExternalInput").ap()

    out_d = nc.dram_tensor("out_t", [P, HKT, B * Q], F32, kind="ExternalOutput").ap()
    nk_d = nc.dram_tensor("newk", [P, B, Q], F32, kind="ExternalOutput").ap()
    nv_d = nc.dram_tensor("newv", [P, B, Q], F32, kind="ExternalOutput").ap()

    with tile.TileContext(nc) as tc:
        with (
            tc.tile_pool(name="singles", bufs=1) as singles,
            tc.tile_pool(name="hidp", bufs=10) as hidp,
            tc.tile_pool(name="expp", bufs=3) as expp,
            tc.tile_pool(name="smallp", bufs=4) as smallp,
            tc.tile_pool(name="outp", bufs=4) as outp,
            tc.tile_pool(name="psq", bufs=1, space="PSUM") as psq,
            tc.tile_pool(name="psm", bufs=4, space="PSUM") as psm,
        ):
            # ---- resident tensors ----
            # projection weights split into interleaved ktile groups so the
            # first matmuls unblock within a couple of microseconds
            qws, kws, vws = [], [], []
            kall = singles.tile([P, B, T], BF16)           # [d, b, t_k]
            vall = singles.tile([P, B, T // P, HD], BF16)  # [t_in_tile, b, jt, d]
            for s in range(HKT // QWS):
                tq = singles.tile([P, QWS, NHC * HD], BF16, tag=f"qw{s}")
                nc.gpsimd.dma_start(tq[:], qw_d[:, ds(s * QWS, QWS), :])
                qws.append(tq)
                tk = singles.tile([P, QWS, HD], BF16, tag=f"kw{s}")
                nc.gpsimd.dma_start(tk[:], kw_d[:, ds(s * QWS, QWS), :])
                kws.append(tk)
                tv = singles.tile([P, QWS, HD], BF16, tag=f"vw{s}")
                nc.gpsimd.dma_start(tv[:], vw_d[:, ds(s * QWS, QWS), :])
                vws.append(tv)
                if s == 0:
                    # b=0 past K/V land before the first attention needs them
                    nc.gpsimd.dma_start(kall[:, 0, 0:PAST], pk_d[:, 0, :])
                    nc.gpsimd.dma_start(
                        vall[:, 0, 0 : PAST // P, :], pv_d[:, 0, :, :]
                    )
                if s == 2:
                    nc.gpsimd.dma_start(kall[:, 1, 0:PAST], pk_d[:, 1, :])
                    nc.gpsimd.dma_start(
                        vall[:, 1, 0 : PAST // P, :], pv_d[:, 1, :, :]
                    )

            cos2 = singles.tile([P, Q], F32)
            nc.gpsimd.dma_start(cos2[:], cos_d[:])
            sinn = singles.tile([P, Q], F32)
            nc.gpsimd.dma_start(sinn[:], sin_d[:])
            tri = singles.tile([P, P], BF16)
            nc.gpsimd.dma_start(tri[:], tri_d[:])
            idn = singles.tile([P, P], BF16)
            nc.gpsimd.dma_start(idn[:], idn_d[:])
            ones = singles.tile([P, P], BF16)
            nc.vector.memset(ones[:], 1.0)

            # o_proj weights last on the queue (needed latest)
            ow = singles.tile([P, NHC, HID], BF16)
            nc.gpsimd.dma_start(ow[:], ow_d[:])

            qall = singles.tile([P, NHC, B * Q], BF16)     # [d, h, t]
            ctxall = singles.tile([P, B, NHC, Q], BF16)    # [d, b, h, t]

            def rope(psrc, csl, t1tag):
                """rope on a [128(d), CH] psum tile -> two fp32 sbuf halves."""
                t1 = smallp.tile([P, CH], F32, tag=t1tag + "a")
                t2 = smallp.tile([P, CH], F32, tag=t1tag + "b")
                nc.vector.tensor_tensor(t1[:], psrc, cos2[:, csl], mybir.AluOpType.mult)
                nc.vector.tensor_tensor(
                    t2[0:64, :], psrc[64:128, :], sinn[0:64, csl], mybir.AluOpType.mult
                )
                nc.vector.tensor_tensor(
                    t2[64:128, :], psrc[0:64, :], sinn[64:128, csl], mybir.AluOpType.mult
                )
                return t1, t2

            # ---- per-chunk projections + rope + kv append ----
            def proj_chunk(c):
                b, sc = divmod(c, 2)
                csl = ds(sc * CH, CH)        # slice into cos/sin (position axis)
                tsl = ds(c * CH, CH)         # slice into b-major token axis
                qps = psq.tile([P, NHC, CH], F32)
                kps = psm.tile([P, CH], F32, tag="ps512")
                vps = psm.tile([P, CH], F32, tag="ps512")
                for k in range(HKT):
                    ht = hidp.tile([P, CH], BF16)
                    nc.sync.dma_start(ht[:], hid_d[:, k, tsl])
                    st, sp = (k == 0), (k == HKT - 1)
                    s, ks = divmod(k, QWS)
                    for m in range(NHC):
                        nc.tensor.matmul(
                            qps[:, m, :], qws[s][:, ks, ds(m * P, P)], ht[:],
                            start=st, stop=sp,
                        )
                    nc.tensor.matmul(kps[:], kws[s][:, ks, :], ht[:], start=st, stop=sp)
                    nc.tensor.matmul(vps[:], vws[s][:, ks, :], ht[:], start=st, stop=sp)

                # epilogue, ordered to unblock the PE stream fastest:
                # 1) vdt cast (gates the v transposes on the PE)
                vdt = smallp.tile([P, CH], BF16, tag="vdt")
                nc.vector.tensor_copy(vdt[:], vps[:])
                for r in range(CH // P):
                    tp = psm.tile([P, CH], BF16, tag="ps512")
                    nc.tensor.transpose(tp[:, 0:P], vdt[:, ds(r * P, P)], idn[:])
                    jt = (PAST // P) + sc * (CH // P) + r
                    nc.vector.tensor_copy(vall[:, b, jt, :], tp[:, 0:P])
                # 2) rope(q) head 0 (gates the first attention scores)
                for m in range(NHC):
                    t1, t2 = rope(qps[:, m, :], csl, "rq")
                    nc.vector.tensor_tensor(
                        qall[:, m, tsl], t1[:], t2[:], mybir.AluOpType.add
                    )
                    if m == 0:
                        # 3) rope(k) (gates the diagonal scores via kall)
                        t1, t2 = rope(kps[:], csl, "rk")
                        knew = outp.tile([P, CH], F32, tag="of32")
                        nc.vector.tensor_tensor(
                            knew[:], t1[:], t2[:], mybir.AluOpType.add
                        )
                        nc.scalar.copy(kall[:, b, ds(PAST + sc * CH, CH)], knew[:])
                        nc.scalar.dma_start(nk_d[:, b, csl], knew[:])
                        # v fp32 output copy on the scalar engine
                        vnew = outp.tile([P, CH], F32, tag="of32")
                        nc.scalar.copy(vnew[:], vps[:])
                        nc.scalar.dma_start(nv_d[:, b, csl], vnew[:])

            # ---- attention for one (b, h, q-chunk) ----
            def attn(b, h, ch):
                c = 2 * b + ch
                # visible key tiles: (t_k start, q offset in chunk, diagonal?)
                kts = [(128 * j, 0, False) for j in range(PAST // P)]
                kts += [(PAST + 128 * j, 0, False) for j in range(4 * ch)]
                kts += [
                    (PAST + 128 * (4 * ch + r), 128 * r, True) for r in range(4)
                ]
                nkt = len(kts)
                ctxps = psm.tile([P, CH], F32, tag="ps512")
                sumps = psm.tile([P, CH], F32, tag="ps512")
                for idx, (kt0, qoff, diag) in enumerate(kts):
                    n = CH - qoff
                    scps = psm.tile([P, CH], F32, tag="ps512")
                    nc.tensor.matmul(
                        scps[:, 0:n],
                        kall[:, b, ds(kt0, P)],
                        qall[:, h, ds(c * CH + qoff, n)],
                        start=True, stop=True,
                    )
                    et = expp.tile([P, CH], BF16)
                    nc.scalar.activation(
                        et[:, 0:n], scps[:, 0:n],
                        mybir.ActivationFunctionType.Exp, scale=SCALE,
                    )
                    if diag:
                        nc.vector.tensor_tensor(
                            et[:, 0:P], et[:, 0:P], tri[:], mybir.AluOpType.mult
                        )
                    st, sp = (idx == 0), (idx == nkt - 1)
                    nc.tensor.matmul(
                        ctxps[:, ds(qoff, n)], vall[:, b, kt0 // P, :], et[:, 0:n],
                        start=st, stop=sp, skip_group_check=True,
                    )
                    nc.tensor.matmul(
                        sumps[:, ds(qoff, n)], ones[:], et[:, 0:n],
                        start=st, stop=sp, skip_group_check=True,
                    )
                rec = smallp.tile([P, CH], F32, tag="recip")
                nc.vector.reciprocal_approx_fast(rec[:], sumps[:])
                nc.vector.tensor_tensor(
                    ctxall[:, b, h, ds(ch * CH, CH)], ctxps[:], rec[:],
                    mybir.AluOpType.mult,
                )

            # ---- o_proj partial for one (b, hid-tile, q-chunk) ----
            def oproj(b, m, ch):
                ops = psm.tile([P, CH], F32, tag="ps512")
                for h in range(NHC):
                    nc.tensor.matmul(
                        ops[:], ow[:, h, ds(m * P, P)],
                        ctxall[:, b, h, ds(ch * CH, CH)],
                        start=(h == 0), stop=(h == NHC - 1),
                    )
                ot = outp.tile([P, CH], F32, tag="of32")
                if m % 2 == 0:
                    nc.scalar.copy(ot[:], ops[:])
                else:
                    nc.vector.tensor_copy(ot[:], ops[:])
                nc.sync.dma_start(out_d[:, m, ds(b * Q + ch * CH, CH)], ot[:])

            for b in range(B):
                proj_chunk(2 * b + 0)
                for h in range(NHC):
                    attn(b, h, 0)
                proj_chunk(2 * b + 1)
                # first half of oproj(ch0) fills the rope-q wait before the
                # ch1 scores; the rest fills the ch1 softmax-normalize tail
                for m in range(HKT // 2):
                    oproj(b, m, 0)
                for h in range(NHC):
                    attn(b, h, 1)
                for m in range(HKT // 2, HKT):
                    oproj(b, m, 0)
                for m in range(HKT):
                    oproj(b, m, 1)

    nc.compile()
    return nc


_NC = None


def _get_nc():
    global _NC
    if _NC is None:
        _NC = build_nc()
    return _NC


def _to_bf16(a):
    return np.ascontiguousarray(a).astype(NP_BF16)


def prep_in_maps(inputs):
    hs = np.asarray(inputs["hidden_states"], np.float32)
    pos = np.asarray(inputs["position_ids"]).astype(np.int64)
    cos = np.asarray(inputs["rope_cos_freqs"], np.float32)
    sin = np.asarray(inputs["rope_sin_freqs"], np.float32)
    pk = np.asarray(inputs["past_key"], np.float32)
    pv = np.asarray(inputs["past_value"], np.float32)
    q_w = np.asarray(inputs["q_w"], np.float32)
    k_w = np.asarray(inputs["k_w"], np.float32)
    v_w = np.asarray(inputs["v_w"], np.float32)
    o_w = np.asarray(inputs["o_w"], np.float32)

    # hidden [B,Q,HID] -> [p, hid_tile, b-major t]
    hid_t = hs.reshape(B * Q, HID).T.reshape(HKT, P, B * Q).transpose(1, 0, 2)
    hid_t = _to_bf16(hid_t)

    # rope rows for the query positions (same for both batch rows)
    cs = cos[pos[0]].T  # [64, Q]
    sn = sin[pos[0]].T
    cos2 = np.concatenate([cs, cs], axis=0).astype(np.float32)
    sinn = np.concatenate([-sn, sn], axis=0).astype(np.float32)
    tri = np.triu(np.ones((P, P), np.float32)).astype(NP_BF16)
    idn = np.eye(P, dtype=np.float32).astype(NP_BF16)

    in_maps = []
    for c in range(NCORES):
        qw = q_w[512 * c : 512 * (c + 1)]                 # [512, HID]
        kwc = k_w[HD * c : HD * (c + 1)]                  # [128, HID]
        vwc = v_w[HD * c : HD * (c + 1)]
        owc = o_w[:, 512 * c : 512 * (c + 1)]             # [HID, 512]
        in_maps.append(
            {
                "hid_t": hid_t,
                "q_wt": _to_bf16(qw.T.reshape(HKT, P, NHC * HD).transpose(1, 0, 2)),
                "k_wt": _to_bf16(kwc.T.reshape(HKT, P, HD).transpose(1, 0, 2)),
                "v_wt": _to_bf16(vwc.T.reshape(HKT, P, HD).transpose(1, 0, 2)),
                "o_wt": _to_bf16(owc.T.reshape(NHC, P, HID).transpose(1, 0, 2)),
                "pk_t": _to_bf16(pk[:, c].transpose(0, 2, 1).transpose(1, 0, 2)),
                "pv_t": _to_bf16(
                    pv[:, c].reshape(B, PAST // P, P, HD).transpose(2, 0, 1, 3)
                ),
                "cos2": cos2,
                "sinn": sinn,
                "tri": tri,
                "idn": idn,
            }
        )
    return in_maps


def assemble(results, inputs):
    pk = np.asarray(inputs["past_key"], np.float32)
    pv = np.asarray(inputs["past_value"], np.float32)

    out = np.zeros((P, HKT, B * Q), np.float64)
    for r in results:
        out += r["out_t"].astype(np.float64)
    attn_out = (
        out.transpose(1, 0, 2).reshape(HID, B * Q).T.reshape(B, Q, HID)
    ).astype(np.float32)

    newk = np.stack([r["newk"] for r in results], axis=0)  # [8, 128, B, Q]
    newv = np.stack([r["newv"] for r in results], axis=0)
    newk = newk.transpose(2, 0, 3, 1)  # [B, 8, Q, 128]
    newv = newv.transpose(2, 0, 3, 1)
    present_key = np.concatenate([pk, newk], axis=2).astype(np.float32)
    present_value = np.concatenate([pv, newv], axis=2).astype(np.float32)
    return attn_out, present_key, present_value


def run_cores(inputs, trace=False, **kwargs):
    nc = _get_nc()
    in_maps = prep_in_maps(inputs)
    res = bass_utils.run_bass_kernel_spmd(
        nc, in_maps, core_ids=list(range(NCORES)), trace=trace, **kwargs
    )
    return res


def kernel(**inputs):
    res = run_cores(inputs)
    return assemble(res.results, inputs)


# revision 17
# speedup vs baseline: 1.0395x; 1.0395x over previous
"""GQA attention layer (32 Q heads / 8 KV heads / head_dim 128) with RoPE and
KV cache, tensor-parallel over heads across 8 TRN2 NeuronCores.

Sharding: core c owns Q heads 4c..4c+3 and KV head c.  Each core computes its
q/k/v projections, RoPE, causal flash-style attention against its KV-cache
shard, and a partial o_proj (input-dim sharded).  The o_proj partials are
summed host-side (the unshard step for partial-sum sharding), so the device
graph needs no collectives.

All matmuls run in bf16 with fp32 PSUM accumulation.  Layouts are arranged
host-side so every DMA is partition-major contiguous:
  - activations/weights are pre-transposed so the contraction dim (hid / head
    dim / key position) lands on SBUF partitions,
  - scores are computed transposed ([t_k, t_q]) so the exp'd scores feed the
    ctx matmul directly with no on-chip transposes of the prob matrix; the
    softmax denominator comes from an all-ones stationary matmul, and the
    bounded-logit range (|logit| < ~15) makes the max-subtraction unnecessary.
"""

import numpy as np
import ml_dtypes

import concourse.bass as bass
import concourse.tile as tile
from concourse import bacc, mybir
from concourse import bass_utils
from concourse.bass import ds

BF16 = mybir.dt.bfloat16
F32 = mybir.dt.float32
NP_BF16 = ml_dtypes.bfloat16

P = 128
B, Q, PAST = 2, 1024, 1024
T = Q + PAST
HID, NH, NKV, HD = 4096, 32, 8, 128
NCORES = 8
NHC = NH // NCORES          # q heads per core
HKT = HID // P              # hid contraction tiles
CH = 512                    # token chunk (matmul free dim)
NCH = (B * Q) // CH         # 4 chunks, b-major token order
SCALE = float(HD) ** -0.5
QWS = 4                     # qw DMA split granularity (ktiles per DMA tile)


def build_nc():
    nc = bacc.Bacc("TRN2", target_bir_lowering=False, debug=False)

    hid_d = nc.dram_tensor("hid_t", [P, HKT, B * Q], BF16, kind="ExternalInput").ap()
    qw_d = nc.dram_tensor("q_wt", [P, HKT, NHC * HD], BF16, kind="ExternalInput").ap()
    kw_d = nc.dram_tensor("k_wt", [P, HKT, HD], BF16, kind="ExternalInput").ap()
    vw_d = nc.dram_tensor("v_wt", [P, HKT, HD], BF16, kind="ExternalInput").ap()
    ow_d = nc.dram_tensor("o_wt", [P, NHC, HID], BF16, kind="ExternalInput").ap()
    pk_d = nc.dram_tensor("pk_t", [P, B, PAST], BF16, kind="ExternalInput").ap()
    pv_d = nc.dram_tensor("pv_t", [P, B, PAST // P, HD], BF16, kind="ExternalInput").ap()
    cos_d = nc.dram_tensor("cos2", [P, Q], F32, kind="ExternalInput").ap()
    sin_d = nc.dram_tensor("sinn", [P, Q], F32, kind="ExternalInput").ap()
    tri_d = nc.dram_tensor("tri", [P, P], BF16, kind="ExternalInput").ap()
    idn_d = nc.dram_tensor("idn", [P, P], BF16, kind="ExternalInput").ap()

    out_d = nc.dram_tensor("out_t", [P, HKT, B * Q], F32, kind="ExternalOutput").ap()
    nk_d = nc.dram_tensor("newk", [P, B, Q], F32, kind="ExternalOutput").ap()
    nv_d = nc.dram_tensor("newv", [P, B, Q], F32, kind="ExternalOutput").ap()

    with tile.TileContext(nc) as tc:
        with (
            tc.tile_pool(name="singles", bufs=1) as singles,
            tc.tile_pool(name="hidp", bufs=10) as hidp,
            tc.tile_pool(name="expp", bufs=3) as expp,
            tc.tile_pool(name="smallp", bufs=4) as smallp,
            tc.tile_pool(name="outp", bufs=4) as outp,
            tc.tile_pool(name="sump", bufs=2) as sump,
            tc.tile_pool(name="psq", bufs=1, space="PSUM") as psq,
            tc.tile_pool(name="psm", bufs=4, space="PSUM") as psm,
        ):
            # ---- resident tensors ----
            # projection weights split into interleaved ktile groups so the
            # first matmuls unblock within a couple of microseconds
            qws, kws, vws = [], [], []
            kall = singles.tile([P, B, T], BF16)           # [d, b, t_k]
            vall = singles.tile([P, B, T // P, HD], BF16)  # [t_in_tile, b, jt, d]
            for s in range(HKT // QWS):
                tq = singles.tile([P, QWS, NHC * HD], BF16, tag=f"qw{s}")
                nc.gpsimd.dma_start(tq[:], qw_d[:, ds(s * QWS, QWS), :])
                qws.append(tq)
                tk = singles.tile([P, QWS, HD], BF16, tag=f"kw{s}")
                nc.gpsimd.dma_start(tk[:], kw_d[:, ds(s * QWS, QWS), :])
                kws.append(tk)
                tv = singles.tile([P, QWS, HD], BF16, tag=f"vw{s}")
                nc.gpsimd.dma_start(tv[:], vw_d[:, ds(s * QWS, QWS), :])
                vws.append(tv)
                if s == 0:
                    # b=0 past K/V land before the first attention needs them
                    nc.gpsimd.dma_start(kall[:, 0, 0:PAST], pk_d[:, 0, :])
                    nc.gpsimd.dma_start(
                        vall[:, 0, 0 : PAST // P, :], pv_d[:, 0, :, :]
                    )
                if s == 2:
                    nc.gpsimd.dma_start(kall[:, 1, 0:PAST], pk_d[:, 1, :])
                    nc.gpsimd.dma_start(
                        vall[:, 1, 0 : PAST // P, :], pv_d[:, 1, :, :]
                    )

            cos2 = singles.tile([P, Q], F32)
            nc.gpsimd.dma_start(cos2[:], cos_d[:])
            sinn = singles.tile([P, Q], F32)
            nc.gpsimd.dma_start(sinn[:], sin_d[:])
            tri = singles.tile([P, P], BF16)
            nc.gpsimd.dma_start(tri[:], tri_d[:])
            idn = singles.tile([P, P], BF16)
            nc.gpsimd.dma_start(idn[:], idn_d[:])
            ones = singles.tile([P, P], BF16)
            nc.vector.memset(ones[:], 1.0)

            # o_proj weights last on the queue (needed latest)
            ow = singles.tile([P, NHC, HID], BF16)
            nc.gpsimd.dma_start(ow[:], ow_d[:])

            qall = singles.tile([P, NHC, B * Q], BF16)     # [d, h, t]
            ctxall = singles.tile([P, B, NHC, Q], BF16)    # [d, b, h, t]

            def rope(psrc, csl, t1tag):
                """rope on a [128(d), CH] psum tile -> two fp32 sbuf halves."""
                t1 = smallp.tile([P, CH], F32, tag=t1tag + "a")
                t2 = smallp.tile([P, CH], F32, tag=t1tag + "b")
                nc.vector.tensor_tensor(t1[:], psrc, cos2[:, csl], mybir.AluOpType.mult)
                nc.vector.tensor_tensor(
                    t2[0:64, :], psrc[64:128, :], sinn[0:64, csl], mybir.AluOpType.mult
                )
                nc.vector.tensor_tensor(
                    t2[64:128, :], psrc[0:64, :], sinn[64:128, csl], mybir.AluOpType.mult
                )
                return t1, t2

            # ---- per-chunk projections + rope + kv append ----
            def proj_chunk(c):
                b, sc = divmod(c, 2)
                csl = ds(sc * CH, CH)        # slice into cos/sin (position axis)
                tsl = ds(c * CH, CH)         # slice into b-major token axis
                qps = psq.tile([P, NHC, CH], F32)
                kps = psm.tile([P, CH], F32, tag="ps512")
                vps = psm.tile([P, CH], F32, tag="ps512")
                for k in range(HKT):
                    ht = hidp.tile([P, CH], BF16)
                    nc.sync.dma_start(ht[:], hid_d[:, k, tsl])
                    st, sp = (k == 0), (k == HKT - 1)
                    s, ks = divmod(k, QWS)
                    for m in range(NHC):
                        nc.tensor.matmul(
                            qps[:, m, :], qws[s][:, ks, ds(m * P, P)], ht[:],
                            start=st, stop=sp,
                        )
                    nc.tensor.matmul(kps[:], kws[s][:, ks, :], ht[:], start=st, stop=sp)
                    nc.tensor.matmul(vps[:], vws[s][:, ks, :], ht[:], start=st, stop=sp)

                # epilogue, ordered to unblock the PE stream fastest:
                # 1) vdt cast (gates the v transposes on the PE)
                vdt = smallp.tile([P, CH], BF16, tag="vdt")
                nc.vector.tensor_copy(vdt[:], vps[:])
                for r in range(CH // P):
                    tp = psm.tile([P, CH], BF16, tag="ps512")
                    nc.tensor.transpose(tp[:, 0:P], vdt[:, ds(r * P, P)], idn[:])
                    jt = (PAST // P) + sc * (CH // P) + r
                    nc.vector.tensor_copy(vall[:, b, jt, :], tp[:, 0:P])
                # 2) rope(q) head 0 (gates the first attention scores)
                for m in range(NHC):
                    t1, t2 = rope(qps[:, m, :], csl, "rq")
                    nc.vector.tensor_tensor(
                        qall[:, m, tsl], t1[:], t2[:], mybir.AluOpType.add
                    )
                    if m == 0:
                        # 3) rope(k) (gates the diagonal scores via kall)
                        t1, t2 = rope(kps[:], csl, "rk")
                        knew = outp.tile([P, CH], F32, tag="of32")
                        nc.vector.tensor_tensor(
                            knew[:], t1[:], t2[:], mybir.AluOpType.add
                        )
                        nc.scalar.copy(kall[:, b, ds(PAST + sc * CH, CH)], knew[:])
                        nc.scalar.dma_start(nk_d[:, b, csl], knew[:])
                        # v fp32 output copy on the scalar engine
                        vnew = outp.tile([P, CH], F32, tag="of32")
                        nc.scalar.copy(vnew[:], vps[:])
                        nc.scalar.dma_start(nv_d[:, b, csl], vnew[:])

            # ---- attention for one (b, h, q-chunk) ----
            def attn(b, h, ch):
                c = 2 * b + ch
                # visible key tiles: (t_k start, q offset in chunk, diagonal?)
                kts = [(128 * j, 0, False) for j in range(PAST // P)]
                kts += [(PAST + 128 * j, 0, False) for j in range(4 * ch)]
                kts += [
                    (PAST + 128 * (4 * ch + r), 128 * r, True) for r in range(4)
                ]
                nkt = len(kts)
                ctxps = psm.tile([P, CH], F32, tag="ps512")
                sumps = psm.tile([P, CH], F32, tag="ps512")
                # ktiles processed in pairs; the two denominator matmuls of a
                # pair go to disjoint 64-wide PE column groups back-to-back, so
                # they run concurrently (~N cycles for both instead of 2N).
                for pi in range(nkt // 2):
                    ets = []
                    for half in (0, 1):
                        idx = 2 * pi + half
                        kt0, qoff, diag = kts[idx]
                        n = CH - qoff
                        scps = psm.tile([P, CH], F32, tag="ps512")
                        nc.tensor.matmul(
                            scps[:, 0:n],
                            kall[:, b, ds(kt0, P)],
                            qall[:, h, ds(c * CH + qoff, n)],
                            start=True, stop=True,
                        )
                        et = expp.tile([P, CH], BF16)
                        nc.scalar.activation(
                            et[:, 0:n], scps[:, 0:n],
                            mybir.ActivationFunctionType.Exp, scale=SCALE,
                        )
                        if diag:
                            nc.vector.tensor_tensor(
                                et[:, 0:P], et[:, 0:P], tri[:],
                                mybir.AluOpType.mult,
                            )
                        nc.tensor.matmul(
                            ctxps[:, ds(qoff, n)], vall[:, b, kt0 // P, :],
                            et[:, 0:n],
                            start=(idx == 0), stop=(idx == nkt - 1),
                            skip_group_check=True,
                        )
                        ets.append(et)
                    for half in (0, 1):
                        idx = 2 * pi + half
                        _, qoff, _ = kts[idx]
                        n = CH - qoff
                        nc.tensor.matmul(
                            sumps[ds(64 * half, 64), ds(qoff, n)],
                            ones[:, 0:64], ets[half][:, 0:n],
                            start=(pi == 0), stop=(pi == nkt // 2 - 1),
                            skip_group_check=True,
                            tile_position=(0, 64 * half),
                        )
                s2 = sump.tile([64, CH], F32, tag="s2")
                nc.vector.tensor_copy(s2[:], sumps[64:128, :])
                sumt = sump.tile([64, CH], F32, tag="sumt")
                nc.vector.tensor_tensor(
                    sumt[:], sumps[0:64, :], s2[:], mybir.AluOpType.add
                )
                rec = sump.tile([64, CH], F32, tag="recip")
                nc.vector.reciprocal_approx_fast(rec[:], sumt[:])
                tsl2 = ds(ch * CH, CH)
                nc.vector.tensor_tensor(
                    ctxall[0:64, b, h, tsl2], ctxps[0:64, :], rec[:],
                    mybir.AluOpType.mult,
                )
                nc.vector.tensor_tensor(
                    ctxall[64:128, b, h, tsl2], ctxps[64:128, :], rec[:],
                    mybir.AluOpType.mult,
                )

            # ---- o_proj partial for one (b, hid-tile, q-chunk) ----
            def oproj(b, m, ch):
                ops = psm.tile([P, CH], F32, tag="ps512")
                for h in range(NHC):
                    nc.tensor.matmul(
                        ops[:], ow[:, h, ds(m * P, P)],
                        ctxall[:, b, h, ds(ch * CH, CH)],
                        start=(h == 0), stop=(h == NHC - 1),
                    )
                ot = outp.tile([P, CH], F32, tag="of32")
                if m % 2 == 0:
                    nc.scalar.copy(ot[:], ops[:])
                else:
                    nc.vector.tensor_copy(ot[:], ops[:])
                nc.sync.dma_start(out_d[:, m, ds(b * Q + ch * CH, CH)], ot[:])

            for b in range(B):
                proj_chunk(2 * b + 0)
                for h in range(NHC):
                    attn(b, h, 0)
                proj_chunk(2 * b + 1)
                # first half of oproj(ch0) fills the rope-q wait before the
                # ch1 scores; the rest fills the ch1 softmax-normalize tail
                for m in range(HKT // 2):
                    oproj(b, m, 0)
                for h in range(NHC):
                    attn(b, h, 1)
                for m in range(HKT // 2, HKT):
                    oproj(b, m, 0)
                for m in range(HKT):
                    oproj(b, m, 1)

    nc.compile()
    return nc


_NC = None


def _get_nc():
    global _NC
    if _NC is None:
        _NC = build_nc()
    return _NC


def _to_bf16(a):
    return np.ascontiguousarray(a).astype(NP_BF16)


def prep_in_maps(inputs):
    hs = np.asarray(inputs["hidden_states"], np.float32)
    pos = np.asarray(inputs["position_ids"]).astype(np.int64)
    cos = np.asarray(inputs["rope_cos_freqs"], np.float32)
    sin = np.asarray(inputs["rope_sin_freqs"], np.float32)
    pk = np.asarray(inputs["past_key"], np.float32)
    pv = np.asarray(inputs["past_value"], np.float32)
    q_w = np.asarray(inputs["q_w"], np.float32)
    k_w = np.asarray(inputs["k_w"], np.float32)
    v_w = np.asarray(inputs["v_w"], np.float32)
    o_w = np.asarray(inputs["o_w"], np.float32)

    # hidden [B,Q,HID] -> [p, hid_tile, b-major t]
    hid_t = hs.reshape(B * Q, HID).T.reshape(HKT, P, B * Q).transpose(1, 0, 2)
    hid_t = _to_bf16(hid_t)

    # rope rows for the query positions (same for both batch rows)
    cs = cos[pos[0]].T  # [64, Q]
    sn = sin[pos[0]].T
    cos2 = np.concatenate([cs, cs], axis=0).astype(np.float32)
    sinn = np.concatenate([-sn, sn], axis=0).astype(np.float32)
    tri = np.triu(np.ones((P, P), np.float32)).astype(NP_BF16)
    idn = np.eye(P, dtype=np.float32).astype(NP_BF16)

    in_maps = []
    for c in range(NCORES):
        qw = q_w[512 * c : 512 * (c + 1)]                 # [512, HID]
        kwc = k_w[HD * c : HD * (c + 1)]                  # [128, HID]
        vwc = v_w[HD * c : HD * (c + 1)]
        owc = o_w[:, 512 * c : 512 * (c + 1)]             # [HID, 512]
        in_maps.append(
            {
                "hid_t": hid_t,
                "q_wt": _to_bf16(qw.T.reshape(HKT, P, NHC * HD).transpose(1, 0, 2)),
                "k_wt": _to_bf16(kwc.T.reshape(HKT, P, HD).transpose(1, 0, 2)),
                "v_wt": _to_bf16(vwc.T.reshape(HKT, P, HD).transpose(1, 0, 2)),
                "o_wt": _to_bf16(owc.T.reshape(NHC, P, HID).transpose(1, 0, 2)),
                "pk_t": _to_bf16(pk[:, c].transpose(0, 2, 1).transpose(1, 0, 2)),
                "pv_t": _to_bf16(
                    pv[:, c].reshape(B, PAST // P, P, HD).transpose(2, 0, 1, 3)
                ),
                "cos2": cos2,
                "sinn": sinn,
                "tri": tri,
                "idn": idn,
            }
        )
    return in_maps


def assemble(results, inputs):
    pk = np.asarray(inputs["past_key"], np.float32)
    pv = np.asarray(inputs["past_value"], np.float32)

    out = np.zeros((P, HKT, B * Q), np.float64)
    for r in results:
        out += r["out_t"].astype(np.float64)
    attn_out = (
        out.transpose(1, 0, 2).reshape(HID, B * Q).T.reshape(B, Q, HID)
    ).astype(np.float32)

    newk = np.stack([r["newk"] for r in results], axis=0)  # [8, 128, B, Q]
    newv = np.stack([r["newv"] for r in results], axis=0)
    newk = newk.transpose(2, 0, 3, 1)  # [B, 8, Q, 128]
    newv = newv.transpose(2, 0, 3, 1)
    present_key = np.concatenate([pk, newk], axis=2).astype(np.float32)
    present_value = np.concatenate([pv, newv], axis=2).astype(np.float32)
    return attn_out, present_key, present_value


def run_cores(inputs, trace=False, **kwargs):
    nc = _get_nc()
    in_maps = prep_in_maps(inputs)
    res = bass_utils.run_bass_kernel_spmd(
        nc, in_maps, core_ids=list(range(NCORES)), trace=trace, **kwargs
    )
    return res


def kernel(**inputs):
    res = run_cores(inputs)
    return assemble(res.results, inputs)


# revision 20
# speedup vs baseline: 1.1653x; 1.1209x over previous
"""GQA attention layer (32 Q heads / 8 KV heads / head_dim 128) with RoPE and
KV cache, tensor-parallel over heads across 8 TRN2 NeuronCores.

Sharding: core c owns Q heads 4c..4c+3 and KV head c.  Each core computes its
q/k/v projections, RoPE, causal flash-style attention against its KV-cache
shard, and a partial o_proj (input-dim sharded).  The o_proj partials are
summed host-side (the unshard step for partial-sum sharding), so the device
graph needs no collectives.

All matmuls run in bf16 with fp32 PSUM accumulation.  Layouts are arranged
host-side so every DMA is partition-major contiguous:
  - activations/weights are pre-transposed so the contraction dim (hid / head
    dim / key position) lands on SBUF partitions,
  - scores are computed transposed ([t_k, t_q]) so the exp'd scores feed the
    ctx matmul directly with no on-chip transposes of the prob matrix; the
    softmax denominator comes from an all-ones stationary matmul, and the
    bounded-logit range (|logit| < ~15) makes the max-subtraction unnecessary.
"""

import numpy as np
import ml_dtypes

import concourse.bass as bass
import concourse.tile as tile
from concourse import bacc, mybir
from concourse import bass_utils
from concourse.bass import ds

BF16 = mybir.dt.bfloat16
F32 = mybir.dt.float32
NP_BF16 = ml_dtypes.bfloat16

P = 128
B, Q, PAST = 2, 1024, 1024
T = Q + PAST
HID, NH, NKV, HD = 4096, 32, 8, 128
NCORES = 8
NHC = NH // NCORES          # q heads per core
HKT = HID // P              # hid contraction tiles
CH = 512                    # token chunk (matmul free dim)
NCH = (B * Q) // CH         # 4 chunks, b-major token order
SCALE = float(HD) ** -0.5
QWS = 4                     # qw DMA split granularity (ktiles per DMA tile)


def build_nc():
    nc = bacc.Bacc("TRN2", target_bir_lowering=False, debug=False)

    hid_d = nc.dram_tensor("hid_t", [P, HKT, B * Q], BF16, kind="ExternalInput").ap()
    qw_d = nc.dram_tensor("q_wt", [P, HKT, NHC * HD], BF16, kind="ExternalInput").ap()
    kw_d = nc.dram_tensor("k_wt", [P, HKT, HD], BF16, kind="ExternalInput").ap()
    vw_d = nc.dram_tensor("v_wt", [P, HKT, HD], BF16, kind="ExternalInput").ap()
    ow_d = nc.dram_tensor("o_wt", [P, NHC, HID], BF16, kind="ExternalInput").ap()
    pk_d = nc.dram_tensor("pk_t", [P, B, PAST], BF16, kind="ExternalInput").ap()
    pv_d = nc.dram_tensor("pv_t", [P, B, PAST // P, HD], BF16, kind="ExternalInput").ap()
    cos_d = nc.dram_tensor("cos2", [P, Q], F32, kind="ExternalInput").ap()
    sin_d = nc.dram_tensor("sinn", [P, Q], F32, kind="ExternalInput").ap()
    tri_d = nc.dram_tensor("tri", [P, P], BF16, kind="ExternalInput").ap()
    idn_d = nc.dram_tensor("idn", [P, P], BF16, kind="ExternalInput").ap()

    out_d = nc.dram_tensor("out_t", [P, HKT, B * Q], F32, kind="ExternalOutput").ap()
    nk_d = nc.dram_tensor("newk", [P, B, Q], F32, kind="ExternalOutput").ap()
    nv_d = nc.dram_tensor("newv", [P, B, Q], F32, kind="ExternalOutput").ap()

    with tile.TileContext(nc) as tc:
        with (
            tc.tile_pool(name="singles", bufs=1) as singles,
            tc.tile_pool(name="hidp", bufs=10) as hidp,
            tc.tile_pool(name="expp", bufs=3) as expp,
            tc.tile_pool(name="smallp", bufs=4) as smallp,
            tc.tile_pool(name="outp", bufs=4) as outp,
            tc.tile_pool(name="ctxp", bufs=2) as ctxp,
            tc.tile_pool(name="psq", bufs=1, space="PSUM") as psq,
            tc.tile_pool(name="psm", bufs=4, space="PSUM") as psm,
        ):
            # ---- resident tensors ----
            # projection weights split into interleaved ktile groups so the
            # first matmuls unblock within a couple of microseconds
            qws, kws, vws = [], [], []
            kall = singles.tile([P, B, T], BF16)           # [d, b, t_k]
            vall = singles.tile([P, B, T // P, HD], BF16)  # [t_in_tile, b, jt, d]
            for s in range(HKT // QWS):
                tq = singles.tile([P, QWS, NHC * HD], BF16, tag=f"qw{s}")
                nc.gpsimd.dma_start(tq[:], qw_d[:, ds(s * QWS, QWS), :])
                qws.append(tq)
                tk = singles.tile([P, QWS, HD], BF16, tag=f"kw{s}")
                nc.gpsimd.dma_start(tk[:], kw_d[:, ds(s * QWS, QWS), :])
                kws.append(tk)
                tv = singles.tile([P, QWS, HD], BF16, tag=f"vw{s}")
                nc.gpsimd.dma_start(tv[:], vw_d[:, ds(s * QWS, QWS), :])
                vws.append(tv)
                if s == 0:
                    # b=0 past K/V land before the first attention needs them
                    nc.gpsimd.dma_start(kall[:, 0, 0:PAST], pk_d[:, 0, :])
                    nc.gpsimd.dma_start(
                        vall[:, 0, 0 : PAST // P, :], pv_d[:, 0, :, :]
                    )
                if s == 2:
                    nc.gpsimd.dma_start(kall[:, 1, 0:PAST], pk_d[:, 1, :])
                    nc.gpsimd.dma_start(
                        vall[:, 1, 0 : PAST // P, :], pv_d[:, 1, :, :]
                    )

            cos2 = singles.tile([P, Q], F32)
            nc.gpsimd.dma_start(cos2[:], cos_d[:])
            sinn = singles.tile([P, Q], F32)
            nc.gpsimd.dma_start(sinn[:], sin_d[:])
            tri = singles.tile([P, P], BF16)
            nc.gpsimd.dma_start(tri[:], tri_d[:])
            idn = singles.tile([P, P], BF16)
            nc.gpsimd.dma_start(idn[:], idn_d[:])
            ones = singles.tile([P, P], BF16)
            nc.vector.memset(ones[:], 1.0)

            # o_proj weights last on the queue (needed latest)
            ow = singles.tile([P, NHC, HID], BF16)
            nc.gpsimd.dma_start(ow[:], ow_d[:])

            qall = singles.tile([P, NHC, B * Q], BF16)     # [d, h, t]
            ctxall = singles.tile([P, B, NHC, Q], BF16)    # [d, b, h, t]

            def rope(psrc, csl, t1tag):
                """rope on a [128(d), CH] psum tile -> two fp32 sbuf halves."""
                t1 = smallp.tile([P, CH], F32, tag=t1tag + "a")
                t2 = smallp.tile([P, CH], F32, tag=t1tag + "b")
                nc.vector.tensor_tensor(t1[:], psrc, cos2[:, csl], mybir.AluOpType.mult)
                nc.vector.tensor_tensor(
                    t2[0:64, :], psrc[64:128, :], sinn[0:64, csl], mybir.AluOpType.mult
                )
                nc.vector.tensor_tensor(
                    t2[64:128, :], psrc[0:64, :], sinn[64:128, csl], mybir.AluOpType.mult
                )
                return t1, t2

            # ---- per-chunk projections + rope + kv append ----
            def proj_chunk(c):
                b, sc = divmod(c, 2)
                csl = ds(sc * CH, CH)        # slice into cos/sin (position axis)
                tsl = ds(c * CH, CH)         # slice into b-major token axis
                qps = psq.tile([P, NHC, CH], F32)
                kps = psm.tile([P, CH], F32, tag="ps512")
                vps = psm.tile([P, CH], F32, tag="ps512")
                for k in range(HKT):
                    ht = hidp.tile([P, CH], BF16)
                    nc.sync.dma_start(ht[:], hid_d[:, k, tsl])
                    st, sp = (k == 0), (k == HKT - 1)
                    s, ks = divmod(k, QWS)
                    for m in range(NHC):
                        nc.tensor.matmul(
                            qps[:, m, :], qws[s][:, ks, ds(m * P, P)], ht[:],
                            start=st, stop=sp,
                        )
                    nc.tensor.matmul(kps[:], kws[s][:, ks, :], ht[:], start=st, stop=sp)
                    nc.tensor.matmul(vps[:], vws[s][:, ks, :], ht[:], start=st, stop=sp)

                # epilogue, ordered to unblock the PE stream fastest:
                # 1) vdt cast (gates the v transposes on the PE)
                vdt = smallp.tile([P, CH], BF16, tag="vdt")
                nc.vector.tensor_copy(vdt[:], vps[:])
                for r in range(CH // P):
                    tp = psm.tile([P, CH], BF16, tag="ps512")
                    nc.tensor.transpose(tp[:, 0:P], vdt[:, ds(r * P, P)], idn[:])
                    jt = (PAST // P) + sc * (CH // P) + r
                    nc.vector.tensor_copy(vall[:, b, jt, :], tp[:, 0:P])
                # 2) rope(q) head 0 (gates the first attention scores)
                for m in range(NHC):
                    t1, t2 = rope(qps[:, m, :], csl, "rq")
                    nc.vector.tensor_tensor(
                        qall[:, m, tsl], t1[:], t2[:], mybir.AluOpType.add
                    )
                    if m == 0:
                        # 3) rope(k) (gates the diagonal scores via kall)
                        t1, t2 = rope(kps[:], csl, "rk")
                        knew = outp.tile([P, CH], F32, tag="of32")
                        nc.vector.tensor_tensor(
                            knew[:], t1[:], t2[:], mybir.AluOpType.add
                        )
                        nc.scalar.copy(kall[:, b, ds(PAST + sc * CH, CH)], knew[:])
                        nc.scalar.dma_start(nk_d[:, b, csl], knew[:])
                        # v fp32 output copy on the scalar engine
                        vnew = outp.tile([P, CH], F32, tag="of32")
                        nc.scalar.copy(vnew[:], vps[:])
                        nc.scalar.dma_start(nv_d[:, b, csl], vnew[:])

            # ---- attention for one (b, h, q-chunk) ----
            def attn(b, h, ch):
                c = 2 * b + ch
                # visible key tiles: (t_k start, q offset in chunk, diagonal?)
                kts = [(128 * j, 0, False) for j in range(PAST // P)]
                kts += [(PAST + 128 * j, 0, False) for j in range(4 * ch)]
                kts += [
                    (PAST + 128 * (4 * ch + r), 128 * r, True) for r in range(4)
                ]
                nkt = len(kts)
                ctxps = psm.tile([P, CH], F32, tag="ps512")
                sumps = psm.tile([P, CH], F32, tag="ps512")
                for idx, (kt0, qoff, diag) in enumerate(kts):
                    n = CH - qoff
                    scps = psm.tile([P, CH], F32, tag="ps512")
                    nc.tensor.matmul(
                        scps[:, 0:n],
                        kall[:, b, ds(kt0, P)],
                        qall[:, h, ds(c * CH + qoff, n)],
                        start=True, stop=True,
                    )
                    et = expp.tile([P, CH], BF16)
                    nc.scalar.activation(
                        et[:, 0:n], scps[:, 0:n],
                        mybir.ActivationFunctionType.Exp, scale=SCALE,
                    )
                    if diag:
                        nc.vector.tensor_tensor(
                            et[:, 0:P], et[:, 0:P], tri[:], mybir.AluOpType.mult
                        )
                    st, sp = (idx == 0), (idx == nkt - 1)
                    nc.tensor.matmul(
                        ctxps[:, ds(qoff, n)], vall[:, b, kt0 // P, :], et[:, 0:n],
                        start=st, stop=sp, skip_group_check=True,
                    )
                    nc.tensor.matmul(
                        sumps[:, ds(qoff, n)], ones[:], et[:, 0:n],
                        start=st, stop=sp, skip_group_check=True,
                    )
                # free the ctx/sums PSUM slots fast so the next head's
                # accumulators can allocate; normalize from SBUF off the
                # critical path
                ctxf = ctxp.tile([P, CH], F32, tag="ctxf")
                nc.scalar.copy(ctxf[:], ctxps[:])
                rec = smallp.tile([P, CH], F32, tag="recip")
                nc.vector.reciprocal_approx_fast(rec[:], sumps[:])
                nc.vector.tensor_tensor(
                    ctxall[:, b, h, ds(ch * CH, CH)], ctxf[:], rec[:],
                    mybir.AluOpType.mult,
                )

            # ---- o_proj partial for one (b, hid-tile, q-chunk) ----
            def oproj(b, m, ch):
                ops = psm.tile([P, CH], F32, tag="ps512")
                for h in range(NHC):
                    nc.tensor.matmul(
                        ops[:], ow[:, h, ds(m * P, P)],
                        ctxall[:, b, h, ds(ch * CH, CH)],
                        start=(h == 0), stop=(h == NHC - 1),
                    )
                ot = outp.tile([P, CH], F32, tag="of32")
                if m % 2 == 0:
                    nc.scalar.copy(ot[:], ops[:])
                else:
                    nc.vector.tensor_copy(ot[:], ops[:])
                nc.sync.dma_start(out_d[:, m, ds(b * Q + ch * CH, CH)], ot[:])

            for b in range(B):
                proj_chunk(2 * b + 0)
                for h in range(NHC):
                    attn(b, h, 0)
                proj_chunk(2 * b + 1)
                # first half of oproj(ch0) fills the rope-q wait before the
                # ch1 scores; the rest fills the ch1 softmax-normalize tail
                for m in range(HKT // 2):
                    oproj(b, m, 0)
                for h in range(NHC):
                    attn(b, h, 1)
                for m in range(HKT // 2, HKT):
                    oproj(b, m, 0)
                for m in range(HKT):
                    oproj(b, m, 1)

    nc.compile()
    return nc


_NC = None


def _get_nc():
    global _NC
    if _NC is None:
        _NC = build_nc()
    return _NC


def _to_bf16(a):
    return np.ascontiguousarray(a).astype(NP_BF16)


def prep_in_maps(inputs):
    hs = np.asarray(inputs["hidden_states"], np.float32)
    pos = np.asarray(inputs["position_ids"]).astype(np.int64)
    cos = np.asarray(inputs["rope_cos_freqs"], np.float32)
    sin = np.asarray(inputs["rope_sin_freqs"], np.float32)
    pk = np.asarray(inputs["past_key"], np.float32)
    pv = np.asarray(inputs["past_value"], np.float32)
    q_w = np.asarray(inputs["q_w"], np.float32)
    k_w = np.asarray(inputs["k_w"], np.float32)
    v_w = np.asarray(inputs["v_w"], np.float32)
    o_w = np.asarray(inputs["o_w"], np.float32)

    # hidden [B,Q,HID] -> [p, hid_tile, b-major t]
    hid_t = hs.reshape(B * Q, HID).T.reshape(HKT, P, B * Q).transpose(1, 0, 2)
    hid_t = _to_bf16(hid_t)

    # rope rows for the query positions (same for both batch rows)
    cs = cos[pos[0]].T  # [64, Q]
    sn = sin[pos[0]].T
    cos2 = np.concatenate([cs, cs], axis=0).astype(np.float32)
    sinn = np.concatenate([-sn, sn], axis=0).astype(np.float32)
    tri = np.triu(np.ones((P, P), np.float32)).astype(NP_BF16)
    idn = np.eye(P, dtype=np.float32).astype(NP_BF16)

    in_maps = []
    for c in range(NCORES):
        qw = q_w[512 * c : 512 * (c + 1)]                 # [512, HID]
        kwc = k_w[HD * c : HD * (c + 1)]                  # [128, HID]
        vwc = v_w[HD * c : HD * (c + 1)]
        owc = o_w[:, 512 * c : 512 * (c + 1)]             # [HID, 512]
        in_maps.append(
            {
                "hid_t": hid_t,
                "q_wt": _to_bf16(qw.T.reshape(HKT, P, NHC * HD).transpose(1, 0, 2)),
                "k_wt": _to_bf16(kwc.T.reshape(HKT, P, HD).transpose(1, 0, 2)),
                "v_wt": _to_bf16(vwc.T.reshape(HKT, P, HD).transpose(1, 0, 2)),
                "o_wt": _to_bf16(owc.T.reshape(NHC, P, HID).transpose(1, 0, 2)),
                "pk_t": _to_bf16(pk[:, c].transpose(0, 2, 1).transpose(1, 0, 2)),
                "pv_t": _to_bf16(
                    pv[:, c].reshape(B, PAST // P, P, HD).transpose(2, 0, 1, 3)
                ),
                "cos2": cos2,
                "sinn": sinn,
                "tri": tri,
                "idn": idn,
            }
        )
    return in_maps


def assemble(results, inputs):
    pk = np.asarray(inputs["past_key"], np.float32)
    pv = np.asarray(inputs["past_value"], np.float32)

    out = np.zeros((P, HKT, B * Q), np.float64)
    for r in results:
        out += r["out_t"].astype(np.float64)
    attn_out = (
        out.transpose(1, 0, 2).reshape(HID, B * Q).T.reshape(B, Q, HID)
    ).astype(np.float32)

    newk = np.stack([r["newk"] for r in results], axis=0)  # [8, 128, B, Q]
    newv = np.stack([r["newv"] for r in results], axis=0)
    newk = newk.transpose(2, 0, 3, 1)  # [B, 8, Q, 128]
    newv = newv.transpose(2, 0, 3, 1)
    present_key = np.concatenate([pk, newk], axis=2).astype(np.float32)
    present_value = np.concatenate([pv, newv], axis=2).astype(np.float32)
    return attn_out, present_key, present_value


def run_cores(inputs, trace=False, **kwargs):
    nc = _get_nc()
    in_maps = prep_in_maps(inputs)
    res = bass_utils.run_bass_kernel_spmd(
        nc, in_maps, core_ids=list(range(NCORES)), trace=trace, **kwargs
    )
    return res


def kernel(**inputs):
    res = run_cores(inputs)
    return assemble(res.results, inputs)
